# revision 1
# baseline (speedup 1.0000x reference)
"""DistGCN on 8 Trainium2 NeuronCores.

3-layer GraphConv (norm='right'): per layer
    h = feat @ W (pre-transform on layers 0 and 2)
    agg[d] = sum_{e: dst[e]=d} h[src[e]]   (segment_sum over 1.6M edges)
    rst = agg / max(deg,1) + b  (+relu on layers 0, 1)

Distribution (per the sharding hint): nodes and their incident edges
(grouped by dst) are sharded across the 8 cores; the small weight
matrices are replicated; a full AllGather of the transformed node
features runs before each aggregation (for a random graph the halo is
effectively the whole table).

Aggregation: per-core dsts are degree-sorted so each 128-dst tile has
near-uniform in-degree; every dst in tile t gets exactly D[t] gather
slots (a common compile-time schedule across cores; padding slots point
at a row that is guaranteed zero).  Each slot column is fetched with one
[128,1]-offset indirect DMA (one gathered row per partition -- the only
indirect-DMA shape this stack executes correctly), and a strided
VectorE reduce computes the segment sums.
"""

import numpy as np

# ---- problem constants (hardcoded per the task contract) ----
N_NODES = 100000
N_EDGES = 1600000
N_CORES = 8
F_IN = 256
F_HID = 64
F_OUT = 32
P = 128

NPC = N_NODES // N_CORES          # 12500 real nodes per core
N_TILES = (NPC + P - 1) // P      # 98
NPAD = N_TILES * P                # 12544 padded rows per core

GATHER_SLOT_BUDGET = 176          # max slots per gather chunk (x256B = 44KB/part)
T0_CHUNK = 14                     # tiles per chunk in the dense input phase

# toggles for the test harness
TRACE = False
LAST_EXEC_NS = None
LAST_RESULTS = None

_CACHE = {}


# ======================================================================
# host-side planning
# ======================================================================

def _build_plan(src, dst, n_nodes=N_NODES, n_cores=N_CORES, npc=None):
    """Shard edges by dst core, degree-sort local dsts, build the common
    slot schedule D[t], per-core gather index arrays, deginv and perms."""
    src = np.asarray(src).astype(np.int64).ravel()
    dst = np.asarray(dst).astype(np.int64).ravel()
    if npc is None:
        npc = n_nodes // n_cores
    n_tiles = (npc + P - 1) // P
    npad = n_tiles * P

    deg = np.bincount(dst, minlength=n_nodes).astype(np.int64)

    # per-core degree-sorted permutation of local nodes
    perms = []
    inv_sortpos = np.empty(n_nodes, np.int64)
    tile_max = np.zeros((n_cores, n_tiles), np.int64)
    for c in range(n_cores):
        dl = deg[c * npc:(c + 1) * npc]
        order = np.argsort(-dl, kind="stable")
        perms.append(order)
        inv_sortpos[c * npc + order] = np.arange(npc)
        sdp = np.zeros(npad, np.int64)
        sdp[:npc] = dl[order]
        tile_max[c] = sdp.reshape(n_tiles, P).max(axis=1)

    D = np.maximum(tile_max.max(axis=0), 1)
    SOFF = np.concatenate([[0], np.cumsum(D)]).astype(np.int64)
    S_TOTAL = int(SOFF[-1])

    # table row of global node n (owner c, sorted pos j):
    #   row = c*npad + (j % P)*n_tiles + (j // P)      (p-major within core)
    assert npc < npad, "need at least one padding row for the zero row"
    zero_row = npad - 1  # core 0, partition 127, last tile: always padding

    e_c = (dst // npc).astype(np.int64)
    e_j = inv_sortpos[dst]
    e_t = e_j // P
    e_p = e_j % P
    order_e = np.argsort(dst, kind="stable")
    starts = np.concatenate([[0], np.cumsum(np.bincount(dst, minlength=n_nodes))])
    rank_sorted = np.arange(len(dst)) - starts[dst[order_e]]
    e_k = np.empty(len(dst), np.int64)
    e_k[order_e] = rank_sorted

    sj = inv_sortpos[src]
    src_row = (src // npc) * npad + (sj % P) * n_tiles + (sj // P)

    idx = np.full((n_cores, P, S_TOTAL), zero_row, np.int32)
    col = SOFF[e_t] + e_k
    flat = e_c * (P * S_TOTAL) + e_p * S_TOTAL + col
    idx.reshape(-1)[flat] = src_row.astype(np.int32)

    deginv = np.ones((n_cores, P, n_tiles), np.float32)
    for c in range(n_cores):
        dl = deg[c * npc:(c + 1) * npc].astype(np.float64)
        sd = np.ones(npad, np.float64)
        dsort = dl[perms[c]]
        sd[:npc] = 1.0 / np.maximum(dsort, 1.0)
        deginv[c] = sd.reshape(n_tiles, P).T.astype(np.float32)

    # gather chunks: consecutive tiles with sum(D) <= budget
    chunks = []
    t0 = 0
    while t0 < n_tiles:
        t1 = t0 + 1
        s = D[t0]
        while t1 < n_tiles and s + D[t1] <= GATHER_SLOT_BUDGET:
            s += D[t1]
            t1 += 1
        chunks.append((t0, t1))
        t0 = t1

    return dict(
        n_nodes=n_nodes, n_cores=n_cores, npc=npc, n_tiles=n_tiles, npad=npad,
        tbl=n_cores * npad, D=D, SOFF=SOFF, S_TOTAL=S_TOTAL,
        idx=idx, deginv=deginv, perms=perms, chunks=chunks,
    )


def _pmajor_rows(perm, npc, npad, n_tiles):
    """row r holds sorted position j = (r % n_tiles)*P + r//n_tiles."""
    r = np.arange(npad)
    p = r // n_tiles
    t = r % n_tiles
    j = t * P + p
    valid = j < npc
    node = np.zeros(npad, np.int64)
    node[valid] = perm[j[valid]]
    return node, valid, j


# ======================================================================
# device program
# ======================================================================

def _build_bass(plan, f_in=F_IN, f_hid=F_HID, f_out=F_OUT):
    import concourse.bass as bass
    import concourse.bacc as bacc
    import concourse.tile as tile
    from concourse import mybir
    from concourse.masks import make_identity

    nc_cores = plan["n_cores"]
    n_tiles = plan["n_tiles"]
    npad = plan["npad"]
    tbl = plan["tbl"]
    D = plan["D"]
    SOFF = plan["SOFF"]
    S_TOTAL = plan["S_TOTAL"]
    chunks = plan["chunks"]
    nt_max = max(t1 - t0 for t0, t1 in chunks)
    xbuf_w = max(T0_CHUNK, nt_max) * f_hid
    KT = f_in // P

    f32 = mybir.dt.float32
    nc = bacc.Bacc("TRN2", target_bir_lowering=False, debug=False,
                   num_devices=nc_cores)

    feat = nc.dram_tensor("featT_shard", [f_in, npad], f32, kind="ExternalInput")
    idx_d = nc.dram_tensor("gidx", [P, S_TOTAL], mybir.dt.int32,
                           kind="ExternalInput")
    dinv_d = nc.dram_tensor("deginv", [P, n_tiles], f32, kind="ExternalInput")
    maskl_d = nc.dram_tensor("maskl", [P, 1], f32, kind="ExternalInput")
    W0_d = nc.dram_tensor("W0", [f_in, f_hid], f32, kind="ExternalInput")
    b0_d = nc.dram_tensor("b0", [f_hid], f32, kind="ExternalInput")
    W1_d = nc.dram_tensor("W1", [f_hid, f_hid], f32, kind="ExternalInput")
    b1_d = nc.dram_tensor("b1", [f_hid], f32, kind="ExternalInput")
    W2_d = nc.dram_tensor("W2", [f_hid, f_out], f32, kind="ExternalInput")
    b2_d = nc.dram_tensor("b2", [f_out], f32, kind="ExternalInput")
    out_d = nc.dram_tensor("out", [npad, f_out], f32, kind="ExternalOutput")

    rg = [list(range(nc_cores))]

    with tile.TileContext(nc) as tc:
        with (
            tc.tile_pool(name="const", bufs=1) as cp,
            tc.tile_pool(name="feat", bufs=3) as fp,
            tc.tile_pool(name="tpose", bufs=3) as tp_pool,
            tc.tile_pool(name="chunkout", bufs=2) as xp,
            tc.tile_pool(name="gather", bufs=2) as gp,
            tc.tile_pool(name="small", bufs=4) as sm,
            tc.tile_pool(name="psum", bufs=3, space="PSUM") as pp,
            tc.tile_pool(name="psum_mm", bufs=3, space="PSUM") as pm,
            tc.tile_pool(name="dram", bufs=1, space="DRAM") as dp,
        ):
            # ---------- constants ----------
            ident = cp.tile([P, P], f32)
            make_identity(nc, ident[:])
            idx_t = cp.tile([P, S_TOTAL], mybir.dt.int32)
            nc.sync.dma_start(out=idx_t[:], in_=idx_d[:])
            dinv_t = cp.tile([P, n_tiles], f32)
            nc.sync.dma_start(out=dinv_t[:], in_=dinv_d[:])
            W0_t = [cp.tile([P, f_hid], f32, tag=f"w0_{k}", name=f"w0_{k}")
                    for k in range(KT)]
            for k in range(KT):
                nc.sync.dma_start(out=W0_t[k][:], in_=W0_d[k * P:(k + 1) * P, :])
            W1_t = cp.tile([f_hid, f_hid], f32)
            nc.sync.dma_start(out=W1_t[:], in_=W1_d[:])
            W2_t = cp.tile([f_hid, f_out], f32)
            nc.sync.dma_start(out=W2_t[:], in_=W2_d[:])
            b0_t = cp.tile([P, f_hid], f32)
            nc.sync.dma_start(out=b0_t[:], in_=b0_d[None, :].to_broadcast([P, f_hid]))
            b1_t = cp.tile([P, f_hid], f32)
            nc.sync.dma_start(out=b1_t[:], in_=b1_d[None, :].to_broadcast([P, f_hid]))
            b2_t = cp.tile([P, f_out], f32)
            nc.sync.dma_start(out=b2_t[:], in_=b2_d[None, :].to_broadcast([P, f_out]))
            maskl_t = cp.tile([P, 1], f32)
            nc.sync.dma_start(out=maskl_t[:], in_=maskl_d[:])

            # ---------- DRAM buffers ----------
            bounce0 = dp.tile([npad, f_hid], f32)
            table0 = dp.tile([tbl, f_hid], f32, addr_space="Shared")
            bounce1 = dp.tile([npad, f_hid], f32)
            table1 = dp.tile([tbl, f_hid], f32, addr_space="Shared")
            bounce2 = dp.tile([npad, f_out], f32)
            table2 = dp.tile([tbl, f_out], f32, addr_space="Shared")

            b0_v = bounce0[:].rearrange("(p t) f -> p t f", t=n_tiles)
            b1_v = bounce1[:].rearrange("(p t) f -> p t f", t=n_tiles)
            b2_v = bounce2[:].rearrange("(p t) f -> p t f", t=n_tiles)
            out_v = out_d[:].rearrange("(p t) f -> p t f", t=n_tiles)

            # ============ phase T0: h0 = feat @ W0 (own shard) ============
            t0 = 0
            while t0 < n_tiles:
                t1 = min(t0 + T0_CHUNK, n_tiles)
                nt = t1 - t0
                h0buf = xp.tile([P, xbuf_w], f32, tag="xbuf", name="h0buf")
                fTs = []
                for k in range(KT):
                    fT = fp.tile([P, T0_CHUNK * P], f32, tag=f"fT{k}", name=f"fT{k}")
                    nc.sync.dma_start(
                        out=fT[:, :nt * P],
                        in_=feat[k * P:(k + 1) * P, t0 * P:t1 * P])
                    fTs.append(fT)
                for t in range(t0, t1):
                    hp = pm.tile([P, f_hid], f32, space="PSUM", tag="mm", name="hp")
                    for k in range(KT):
                        nc.tensor.matmul(
                            out=hp[:], lhsT=fTs[k][:, (t - t0) * P:(t - t0 + 1) * P],
                            rhs=W0_t[k][:], start=(k == 0), stop=(k == KT - 1))
                    nc.vector.tensor_copy(
                        out=h0buf[:, (t - t0) * f_hid:(t - t0 + 1) * f_hid],
                        in_=hp[:])
                nc.sync.dma_start(out=b0_v[:, t0:t1, :], in_=h0buf[:, :nt * f_hid])
                t0 = t1

            # ============ AllGather 0 ============
            nc.gpsimd.collective_compute(
                "AllGather", mybir.AluOpType.bypass, replica_groups=rg,
                ins=[bounce0[:]], outs=[table0[:, :]])

            def gather_chunk(table, f_w, ct0, ct1):
                """One [P,1]-offset indirect DMA per slot column."""
                slots = int(SOFF[ct1] - SOFF[ct0])
                gb = gp.tile([P, GATHER_SLOT_BUDGET * f_hid], f32, tag="gb",
                             name="gb")
                for s in range(slots):
                    g = int(SOFF[ct0]) + s
                    nc.gpsimd.indirect_dma_start(
                        out=gb[:, s * f_w:(s + 1) * f_w],
                        out_offset=None,
                        in_=table[:],
                        in_offset=bass.IndirectOffsetOnAxis(
                            ap=idx_t[:, g:g + 1], axis=0),
                    )
                return gb

            def reduce_tile(gb, f_w, ct0, t, dstap):
                o = int(SOFF[t] - SOFF[ct0])
                d = int(D[t])
                seg = gb[:, o * f_w:(o + d) * f_w]
                if d == 1:
                    nc.vector.tensor_copy(out=dstap, in_=seg)
                else:
                    nc.vector.tensor_reduce(
                        out=dstap, in_=seg.rearrange("p (d f) -> p f d", f=f_w),
                        axis=mybir.AxisListType.X, op=mybir.AluOpType.add)

            # ====== phase G0: x1 = relu(agg(h0)*dinv + b0) -> bounce1 ======
            for (ct0, ct1) in chunks:
                nt = ct1 - ct0
                gb = gather_chunk(table0, f_hid, ct0, ct1)
                xb = xp.tile([P, xbuf_w], f32, tag="xbuf", name="xb")
                for t in range(ct0, ct1):
                    reduce_tile(gb, f_hid, ct0, t,
                                xb[:, (t - ct0) * f_hid:(t - ct0 + 1) * f_hid])
                xv = xb[:, :nt * f_hid].rearrange("p (t f) -> p t f", f=f_hid)
                nc.vector.tensor_tensor(
                    out=xv, in0=xv,
                    in1=dinv_t[:, ct0:ct1].rearrange("p t -> p t ()")
                        .to_broadcast([P, nt, f_hid]),
                    op=mybir.AluOpType.mult)
                nc.vector.tensor_tensor(
                    out=xv, in0=xv,
                    in1=b0_t[:].rearrange("p f -> p () f")
                        .to_broadcast([P, nt, f_hid]),
                    op=mybir.AluOpType.add)
                nc.scalar.activation(xb[:, :nt * f_hid], xb[:, :nt * f_hid],
                                     mybir.ActivationFunctionType.Relu)
                if ct1 == n_tiles:
                    zc = (n_tiles - 1 - ct0) * f_hid
                    nc.vector.tensor_scalar(
                        out=xb[:, zc:zc + f_hid], in0=xb[:, zc:zc + f_hid],
                        scalar1=maskl_t[:, 0:1], scalar2=None,
                        op0=mybir.AluOpType.mult)
                nc.sync.dma_start(out=b1_v[:, ct0:ct1, :], in_=xb[:, :nt * f_hid])

            # ============ AllGather 1 ============
            nc.gpsimd.collective_compute(
                "AllGather", mybir.AluOpType.bypass, replica_groups=rg,
                ins=[bounce1[:]], outs=[table1[:, :]])

            # == phase G1: x2 = relu((agg(x1)@W1)*dinv + b1); h2 = x2@W2 ==
            for (ct0, ct1) in chunks:
                nt = ct1 - ct0
                gb = gather_chunk(table1, f_hid, ct0, ct1)
                ob = xp.tile([P, xbuf_w], f32, tag="xbuf", name="ob")
                for t in range(ct0, ct1):
                    agg = sm.tile([P, f_hid], f32, tag="agg", name="agg")
                    reduce_tile(gb, f_hid, ct0, t, agg[:])
                    tps = pp.tile([f_hid, P], f32, space="PSUM", tag="tps",
                                  name="tps")
                    nc.tensor.transpose(out=tps[:], in_=agg[:], identity=ident[:])
                    aT = tp_pool.tile([f_hid, P], f32, tag="aT", name="aT")
                    nc.vector.tensor_copy(out=aT[:], in_=tps[:])
                    mp = pm.tile([P, f_hid], f32, space="PSUM", tag="mm", name="mp")
                    nc.tensor.matmul(out=mp[:], lhsT=aT[:], rhs=W1_t[:],
                                     start=True, stop=True)
                    x2 = sm.tile([P, f_hid], f32, tag="x2", name="x2")
                    nc.vector.tensor_scalar(
                        out=x2[:], in0=mp[:], scalar1=dinv_t[:, t:t + 1],
                        scalar2=None, op0=mybir.AluOpType.mult)
                    nc.vector.tensor_tensor(out=x2[:], in0=x2[:], in1=b1_t[:],
                                            op=mybir.AluOpType.add)
                    nc.scalar.activation(x2[:], x2[:],
                                         mybir.ActivationFunctionType.Relu)
                    if t == n_tiles - 1:
                        nc.vector.tensor_scalar(
                            out=x2[:], in0=x2[:], scalar1=maskl_t[:, 0:1],
                            scalar2=None, op0=mybir.AluOpType.mult)
                    tps2 = pp.tile([f_hid, P], f32, space="PSUM", tag="tps",
                                   name="tps2")
                    nc.tensor.transpose(out=tps2[:], in_=x2[:], identity=ident[:])
                    xT = tp_pool.tile([f_hid, P], f32, tag="xT", name="xT")
                    nc.vector.tensor_copy(out=xT[:], in_=tps2[:])
                    op = pm.tile([P, f_out], f32, space="PSUM", tag="mm", name="op")
                    nc.tensor.matmul(out=op[:], lhsT=xT[:], rhs=W2_t[:],
                                     start=True, stop=True)
                    nc.vector.tensor_copy(
                        out=ob[:, (t - ct0) * f_out:(t - ct0 + 1) * f_out],
                        in_=op[:])
                nc.sync.dma_start(out=b2_v[:, ct0:ct1, :], in_=ob[:, :nt * f_out])

            # ============ AllGather 2 ============
            nc.gpsimd.collective_compute(
                "AllGather", mybir.AluOpType.bypass, replica_groups=rg,
                ins=[bounce2[:]], outs=[table2[:, :]])

            # ====== phase G2: out = agg(h2)*dinv + b2 (no relu) ======
            for (ct0, ct1) in chunks:
                nt = ct1 - ct0
                gb = gather_chunk(table2, f_out, ct0, ct1)
                fb = xp.tile([P, xbuf_w], f32, tag="xbuf", name="fb")
                for t in range(ct0, ct1):
                    reduce_tile(gb, f_out, ct0, t,
                                fb[:, (t - ct0) * f_out:(t - ct0 + 1) * f_out])
                fv = fb[:, :nt * f_out].rearrange("p (t f) -> p t f", f=f_out)
                nc.vector.tensor_tensor(
                    out=fv, in0=fv,
                    in1=dinv_t[:, ct0:ct1].rearrange("p t -> p t ()")
                        .to_broadcast([P, nt, f_out]),
                    op=mybir.AluOpType.mult)
                nc.vector.tensor_tensor(
                    out=fv, in0=fv,
                    in1=b2_t[:].rearrange("p f -> p () f")
                        .to_broadcast([P, nt, f_out]),
                    op=mybir.AluOpType.add)
                nc.sync.dma_start(out=out_v[:, ct0:ct1, :], in_=fb[:, :nt * f_out])

    nc.compile()
    return nc


# ======================================================================
# entry point
# ======================================================================

def _make_in_maps(inputs, plan):
    n_cores = plan["n_cores"]
    npc = plan["npc"]
    npad = plan["npad"]
    n_tiles = plan["n_tiles"]

    feat = np.ascontiguousarray(np.asarray(inputs["feat"], dtype=np.float32))
    f_in = feat.shape[1]

    in_maps = []
    for c in range(n_cores):
        # featT columns are sorted positions j (tile t = cols [t*P,(t+1)*P))
        fs = np.zeros((npad, f_in), np.float32)
        fs[:npc] = feat[c * npc + plan["perms"][c]]
        p_arr = np.arange(P)
        maskl = ((n_tiles - 1) * P + p_arr < npc).astype(np.float32)[:, None]
        in_maps.append({
            "featT_shard": np.ascontiguousarray(fs.T),
            "maskl": maskl,
            "gidx": plan["idx"][c],
            "deginv": plan["deginv"][c],
            "W0": np.asarray(inputs["W0"], np.float32),
            "b0": np.asarray(inputs["b0"], np.float32),
            "W1": np.asarray(inputs["W1"], np.float32),
            "b1": np.asarray(inputs["b1"], np.float32),
            "W2": np.asarray(inputs["W2"], np.float32),
            "b2": np.asarray(inputs["b2"], np.float32),
        })
    return in_maps


def _assemble_out(results, plan):
    npc = plan["npc"]
    npad = plan["npad"]
    n_tiles = plan["n_tiles"]
    f_out = results[0]["out"].shape[-1]
    out = np.empty((plan["n_nodes"], f_out), np.float32)
    for c in range(plan["n_cores"]):
        node, valid, _ = _pmajor_rows(plan["perms"][c], npc, npad, n_tiles)
        shard = results[c]["out"]
        out[c * npc + node[valid]] = shard[valid]
    return out


def _run(inputs, plan, nc):
    from concourse.bass_utils import run_bass_kernel_spmd

    in_maps = _make_in_maps(inputs, plan)
    res = run_bass_kernel_spmd(nc, in_maps, list(range(plan["n_cores"])),
                               trace=TRACE)
    global LAST_EXEC_NS, LAST_RESULTS
    LAST_EXEC_NS = res.exec_time_ns
    LAST_RESULTS = res
    return _assemble_out(res.results, plan)


def kernel(feat, W0, b0, W1, b1, W2, b2, src, dst):
    key = (np.asarray(feat).shape, np.asarray(src).shape)
    if key not in _CACHE:
        plan = _build_plan(src, dst)
        nc = _build_bass(plan)
        _CACHE[key] = (plan, nc)
    plan, nc = _CACHE[key]
    return _run(
        dict(feat=feat, W0=W0, b0=b0, W1=W1, b1=b1, W2=W2, b2=b2),
        plan, nc)

